# revision 32
# baseline (speedup 1.0000x reference)
"""Multi-head causal attention (B=2, T=2048, D=1024, H=16, HD=64) on 8 TRN2
NeuronCores.

Sharding: batch x head-group. Core c handles batch c//4 and heads
[4*(c%4), 4*(c%4)+4). Wq/Wk/Wv are split column-wise, Wo row-wise; each core
produces a full [T, D] partial output (its 4 heads' contribution, after
per-head softmax normalization and its Wo row-block), which the host sums
across the 4 cores of each batch and adds the bias to.

Per-core kernel (all matmuls contract along SBUF partitions; operands bf16,
accumulation fp32 in PSUM):
  xT [D, T] bf16 (host passes x[b].T pre-cast), weights bf16.
  QT/KT computed transposed [2*64hd, T] per head-pair (lhsT = w, rhs = xT).
  V computed natural [T, 4*64hd] (lhsT = xT, rhs = wv), stored bf16 with a
  ones-column per head (stride 66) so the P@V matmul also produces the
  softmax row-sums (M = 65).
  Scores are computed transposed, ST[k, q] (lhsT = KT, rhs = QT), exact-causal
  (q >= 128*kt per k-tile), exp'd on ACT (scale=1/8 fused) to bf16 ET tiles;
  the strictly-lower triangle of the leading 128x128 diagonal block is zeroed
  with a multiplicative mask (gpsimd). CT' = V'.T @ ET accumulates [65, 512]
  per q-chunk in PSUM; partition 64 is the softmax denominator (exact fp32).
  Reciprocal row-sums are broadcast across partitions via a DRAM bounce and
  multiplied into packed bf16 CT_g [128c, T] tiles (DVE), then
  out[t, do] = CT_g.T @ wo (fp16 partial) is DMA'd out.

Emission order keeps the PE dense (HAM warm): QT/KT(g0) upfront, then the
head-serial pipeline ST(h) || CT(h-1), with QT/KT(g1) and V projection units
spread through head 0's score window; CT(h3) trails, then normalize + out.
"""

import contextlib

import numpy as np

T, D = 2048, 1024
NH, HD = 16, 64
HPC = 4  # heads per core
NCORES = 8
ND = D // 128  # 8 d-tiles
NT = T // 128  # 16 t/k-tiles
NQ = T // 512  # 4 q-chunks

_NC = None


def _build_nc():
    import concourse.mybir as mybir
    import concourse.tile as tile
    from concourse import bacc
    from concourse.masks import make_upper_triangular

    f32 = mybir.dt.float32
    bf16 = mybir.dt.bfloat16
    fp16 = mybir.dt.float16
    Exp = mybir.ActivationFunctionType.Exp

    nc = bacc.Bacc("TRN2", target_bir_lowering=False, debug=False, num_devices=NCORES)

    xT_d = nc.dram_tensor("xT", [D, T], bf16, kind="ExternalInput").ap()
    wq_d = nc.dram_tensor("wq", [D, HPC * HD], bf16, kind="ExternalInput").ap()
    wk_d = nc.dram_tensor("wk", [D, HPC * HD], bf16, kind="ExternalInput").ap()
    wv_d = nc.dram_tensor("wv", [D, HPC * HD], bf16, kind="ExternalInput").ap()
    wo_d = nc.dram_tensor("wo", [HPC * HD, D], bf16, kind="ExternalInput").ap()
    out_d = nc.dram_tensor("out", [T, D], fp16, kind="ExternalOutput").ap()
    rscr = nc.dram_tensor("rscr", [128, 64], f32).ap()
    rscr2 = nc.dram_tensor("rscr2", [128, 64], f32).ap()

    with tile.TileContext(nc) as tc, contextlib.ExitStack() as ctx:
        pool = lambda **kw: ctx.enter_context(tc.tile_pool(**kw))
        constp = pool(name="const", bufs=1)
        qkp = pool(name="qk", bufs=1)
        vp = pool(name="vpool", bufs=1)
        wop = pool(name="wop", bufs=1)
        etp = pool(name="et", bufs=2)
        stgp = pool(name="stg", bufs=1)
        ctgp = pool(name="ctg", bufs=1)
        normp = pool(name="norm", bufs=1)
        rbp = pool(name="rb", bufs=4)
        bctx = contextlib.ExitStack()
        psST = bctx.enter_context(tc.tile_pool(name="psST", bufs=2, space="PSUM"))
        psCT = bctx.enter_context(tc.tile_pool(name="psCT", bufs=2, space="PSUM"))
        actx = contextlib.ExitStack()
        apool = lambda **kw: actx.enter_context(tc.tile_pool(**kw))
        xtp = apool(name="xtr", bufs=1)
        wtp = apool(name="wtiles", bufs=1)
        psProj = apool(name="psProj", bufs=2, space="PSUM")

        mask = constp.tile([128, 128], bf16, name="mask")
        make_upper_triangular(nc, mask[:], val=1.0, diag=True)

        QT = [qkp.tile([128, T], bf16, name=f"QT{g}") for g in range(2)]
        KT = [qkp.tile([128, T], bf16, name=f"KT{g}") for g in range(2)]
        vsb = [vp.tile([128, 66 * HPC], bf16, name=f"v{tt}") for tt in range(NT)]
        wo_sb = [wop.tile([128, D], bf16, name=f"wo{gi}") for gi in range(2)]

        # ---------- loads (bf16 straight from DRAM, few big DMAs) ----------
        wtiles = {}
        wsb = {}

        def load_w(wname, wd):
            wsb[wname] = wtp.tile([128, ND * 256], bf16, name=f"{wname}sb", tag=f"{wname}sb")
            nc.sync.dma_start(wsb[wname][:], wd.rearrange("(a p) c -> p a c", p=128))
            wtiles[wname] = [wsb[wname][:, 256 * dt : 256 * (dt + 1)] for dt in range(ND)]

        load_w("wq", wq_d)
        xtr = [xtp.tile([128, T], bf16, name=f"xtr{dt}", tag=f"xtr{dt}") for dt in range(ND)]
        for dt in range(ND):  # first halves: unblock QT/KT chunks 0-1
            nc.sync.dma_start(xtr[dt][:, 0:1024], xT_d[128 * dt : 128 * (dt + 1), 0:1024])
        load_w("wk", wk_d)
        for dt in range(ND):
            nc.sync.dma_start(xtr[dt][:, 1024:T], xT_d[128 * dt : 128 * (dt + 1), 1024:T])
        load_w("wv", wv_d)
        for gi in range(2):
            nc.sync.dma_start(wo_sb[gi][:], wo_d[128 * gi : 128 * (gi + 1), :])

        # ---------- emission units ----------
        def emit_qkt_unit(wname, outs, g, c):
            ps = psProj.tile([128, 512], f32, name=f"pj_{wname}{g}_{c}", tag="proj")
            for dt in range(ND):
                nc.tensor.matmul(
                    ps[:],
                    wtiles[wname][dt][:, 128 * g : 128 * (g + 1)],
                    xtr[dt][:, 512 * c : 512 * (c + 1)],
                    start=(dt == 0),
                    stop=(dt == ND - 1),
                )
            nc.vector.tensor_copy(outs[g][:, 512 * c : 512 * (c + 1)], ps[:])

        def emit_v(tt):
            ps = psProj.tile([128, 256], f32, name=f"vps{tt}", tag="proj")
            for dt in range(ND):
                nc.tensor.matmul(
                    ps[:],
                    xtr[dt][:, 128 * tt : 128 * (tt + 1)],
                    wtiles["wv"][dt][:],
                    start=(dt == 0),
                    stop=(dt == ND - 1),
                )
            nc.any.memset(vsb[tt][:, 64 : 66 * HPC : 66], 1.0)
            for h in range(HPC):
                nc.vector.tensor_copy(vsb[tt][:, 66 * h : 66 * h + 64], ps[:, 64 * h : 64 * (h + 1)])

        ets = {}  # (h, kt) -> ET tile

        def emit_st(h, kt):
            g = h // 2
            p0 = 64 * (h % 2)
            w = T - 128 * kt
            et = etp.tile([128, w], bf16, name=f"et_h{h}_kt{kt}", tag=f"et{kt}")
            ets[(h, kt)] = et
            for sub in range((w + 1023) // 1024):
                sw = min(1024, w - 1024 * sub)
                q0 = 128 * kt + 1024 * sub
                ps = psST.tile([128, sw], f32, name=f"st_h{h}_k{kt}_s{sub}", tag="st")
                for c in range((sw + 511) // 512):
                    n = min(512, sw - 512 * c)
                    nc.tensor.matmul(
                        ps[:, 512 * c : 512 * c + n],
                        KT[g][p0 : p0 + 64, 128 * kt : 128 * (kt + 1)],
                        QT[g][p0 : p0 + 64, q0 + 512 * c : q0 + 512 * c + n],
                        start=True,
                        stop=True,
                    )
                nc.scalar.activation(
                    et[:, 1024 * sub : 1024 * sub + sw], ps[:, 0:sw], Exp, scale=0.125
                )
            nc.gpsimd.tensor_mul(et[:, 0:128], et[:, 0:128], mask[:])

        stg = {}
        ct_ps = {}

        def emit_ct_mms(h, j, kts, first, last):
            if first:
                ct_ps[(h, j)] = psCT.tile([65, 512], f32, name=f"ct_h{h}_j{j}", tag="ct")
            ct = ct_ps[(h, j)]
            nkt = 4 * j + 4
            for kt in kts:
                etoff = 512 * j - 128 * kt
                if etoff >= 0:
                    n, psoff, ecol = 512, 0, etoff
                else:
                    n, psoff, ecol = 512 + etoff, -etoff, 0
                nc.tensor.matmul(
                    ct[0:65, psoff : psoff + n],
                    vsb[kt][:, 66 * h : 66 * h + 65],
                    ets[(h, kt)][:, ecol : ecol + n],
                    start=(kt == 0),
                    stop=(last and kt == kts[-1]),
                )

        def finish_ct(h, j):
            ct = ct_ps[(h, j)]
            s = stgp.tile([65, 512], f32, name=f"stg_h{h}_j{j}")
            stg[(h, j)] = s
            nc.vector.tensor_copy(s[:], ct[:])
            idx = 4 * h + j
            nc.sync.dma_start(rscr[8 * idx : 8 * idx + 8, :], s[64:65, :])

        # normalize pools created up top (before xtr) for stack order
        CTG = [ctgp.tile([128, T], bf16, name=f"ctg{gi}") for gi in range(2)]
        rscr2v = rscr2.rearrange("(r p) c -> r (p c)", p=8)  # [16, 512] view

        def emit_norm(h, j):
            g, half = h // 2, h % 2
            idx = 4 * h + j
            rs_hj = normp.tile([8, 64], f32, name=f"rs{idx}", tag="rs")
            nc.sync.dma_start(rs_hj[:], rscr[8 * idx : 8 * idx + 8, :])
            rc_hj = normp.tile([8, 64], f32, name=f"rc{idx}", tag="rc")
            nc.vector.reciprocal(rc_hj[:], rs_hj[:])
            nc.sync.dma_start(rscr2[8 * idx : 8 * idx + 8, :], rc_hj[:])
            rb = rbp.tile([64, 512], f32, name=f"rb{idx}", tag="rb")
            nc.sync.dma_start(rb[:], rscr2v[idx : idx + 1, :].partition_broadcast(64))
            eng = nc.vector if j % 2 == 0 else nc.gpsimd
            eng.tensor_mul(
                CTG[g][64 * half : 64 * half + 64, 512 * j : 512 * (j + 1)],
                stg[(h, j)][0:64, :],
                rb[:],
            )

        # ---------- schedule ----------
        emit_qkt_unit("wq", QT, 0, 0)
        emit_qkt_unit("wq", QT, 0, 1)
        emit_qkt_unit("wk", KT, 0, 0)
        emit_qkt_unit("wq", QT, 0, 2)
        emit_qkt_unit("wq", QT, 0, 3)

        # head-serial pipeline. Within each head's 16-slot score window,
        # context-matmul groups are dribbled a few MMs per slot so exp is
        # never starved behind a lumpy PE queue: group j<3 spreads over slots
        # 4j+4..4j+7 (j+1 MMs/slot); group j=3 takes one MM per slot from
        # slot 1 (k-tile kt-1), finishing in-window. Each group's normalize
        # chain (rowsum DMA -> packed reciprocal -> DRAM-bounce broadcast ->
        # multiply into CTG) is emitted at its stop slot.
        dribble = {sw: [] for sw in range(NT)}
        for j in range(3):
            kts = list(range(4 * j + 4))
            for sl in range(4):
                chunk = kts[(j + 1) * sl : (j + 1) * (sl + 1)]
                dribble[4 * j + 4 + sl].append((j, chunk, sl == 0, sl == 3))
        for sl in range(1, 15):
            dribble[sl].append((3, [sl - 1], sl == 1, False))
        dribble[15].append((3, [14, 15], False, True))

        for h in range(HPC):
            for sw in range(NT):
                emit_st(h, sw)
                if h == 0:
                    emit_v(sw)
                    if 1 <= sw <= 3:
                        emit_qkt_unit("wk", KT, 0, sw)
                elif h == 1 and sw % 2 == 0:
                    wn, g1c = ("wq", sw // 4) if sw % 4 == 0 else ("wk", sw // 4)
                    emit_qkt_unit(wn, QT if wn == "wq" else KT, 1, g1c)
                for j, kts_, first, last in dribble[sw]:
                    emit_ct_mms(h, j, kts_, first, last)
                    if last:
                        finish_ct(h, j)
                        emit_norm(h, j)
                if h == 1 and sw == NT - 1:
                    actx.close()
        bctx.close()

        # ---------- output projection ----------
        with (
            tc.tile_pool(name="oh", bufs=3) as ohp,
            tc.tile_pool(name="psO", bufs=2, space="PSUM") as psO,
        ):
            for tt in range(NT):
                ps = psO.tile([128, D], f32, name=f"ops{tt}", tag="ops")
                for gi in range(2):
                    for dc in range(2):
                        nc.tensor.matmul(
                            ps[:, 512 * dc : 512 * (dc + 1)],
                            CTG[gi][:, 128 * tt : 128 * (tt + 1)],
                            wo_sb[gi][:, 512 * dc : 512 * (dc + 1)],
                            start=(gi == 0),
                            stop=(gi == 1),
                        )
                oh = ohp.tile([128, D], fp16, name=f"oh{tt}", tag="oh")
                nc.vector.tensor_copy(oh[:], ps[:])
                nc.sync.dma_start(out_d[128 * tt : 128 * (tt + 1), :], oh[:])

    nc.compile()
    return nc


def _get_nc():
    global _NC
    if _NC is None:
        _NC = _build_nc()
    return _NC


def make_in_maps(x, wq, wk, wv, wo):
    import ml_dtypes

    bf = ml_dtypes.bfloat16
    in_maps = []
    for c in range(NCORES):
        b, g4 = c // 4, c % 4
        cs = slice(256 * g4, 256 * (g4 + 1))
        in_maps.append(
            {
                "xT": np.ascontiguousarray(x[b].T).astype(bf),
                "wq": np.ascontiguousarray(wq[:, cs]).astype(bf),
                "wk": np.ascontiguousarray(wk[:, cs]).astype(bf),
                "wv": np.ascontiguousarray(wv[:, cs]).astype(bf),
                "wo": np.ascontiguousarray(wo[cs, :]).astype(bf),
            }
        )
    return in_maps


def kernel(x, wq, wk, wv, wo, bo):
    from concourse.bass_utils import run_bass_kernel_spmd

    x = np.asarray(x, dtype=np.float32)
    wq = np.asarray(wq, dtype=np.float32)
    wk = np.asarray(wk, dtype=np.float32)
    wv = np.asarray(wv, dtype=np.float32)
    wo = np.asarray(wo, dtype=np.float32)
    bo = np.asarray(bo, dtype=np.float32)

    nc = _get_nc()
    in_maps = make_in_maps(x, wq, wk, wv, wo)
    try:
        res = run_bass_kernel_spmd(nc, in_maps, core_ids=list(range(NCORES))).results
    except Exception:
        # transient NRT device errors have been observed once after a fresh
        # compile; one retry recovers
        res = run_bass_kernel_spmd(nc, in_maps, core_ids=list(range(NCORES))).results
    out = np.zeros((2, T, D), dtype=np.float32)
    for c in range(NCORES):
        out[c // 4] += res[c]["out"].astype(np.float32)
    out += bo[None, None, :]
    return out


# revision 33
# speedup vs baseline: 1.0233x; 1.0233x over previous
"""Multi-head causal attention (B=2, T=2048, D=1024, H=16, HD=64) on 8 TRN2
NeuronCores.

Sharding: batch x head-group. Core c handles batch c//4 and heads
[4*(c%4), 4*(c%4)+4). Wq/Wk/Wv are split column-wise, Wo row-wise; each core
produces a full [T, D] partial output (its 4 heads' contribution, after
per-head softmax normalization and its Wo row-block), which the host sums
across the 4 cores of each batch and adds the bias to.

Per-core kernel (all matmuls contract along SBUF partitions; operands bf16,
accumulation fp32 in PSUM):
  xT [D, T] bf16 (host passes x[b].T pre-cast), weights bf16.
  QT/KT computed transposed [2*64hd, T] per head-pair (lhsT = w, rhs = xT).
  V computed natural [T, 4*64hd] (lhsT = xT, rhs = wv), stored bf16 with a
  ones-column per head (stride 66) so the P@V matmul also produces the
  softmax row-sums (M = 65).
  Scores are computed transposed, ST[k, q] (lhsT = KT, rhs = QT), exact-causal
  (q >= 128*kt per k-tile), exp'd on ACT (scale=1/8 fused) to bf16 ET tiles;
  the strictly-lower triangle of the leading 128x128 diagonal block is zeroed
  with a multiplicative mask (gpsimd). CT' = V'.T @ ET accumulates [65, 512]
  per q-chunk in PSUM; partition 64 is the softmax denominator (exact fp32).
  Reciprocal row-sums are broadcast across partitions via a DRAM bounce and
  multiplied into packed bf16 CT_g [128c, T] tiles (DVE), then
  out[t, do] = CT_g.T @ wo (fp16 partial) is DMA'd out.

Emission order keeps the PE dense (HAM warm): QT/KT(g0) upfront, then the
head-serial pipeline ST(h) || CT(h-1), with QT/KT(g1) and V projection units
spread through head 0's score window; CT(h3) trails, then normalize + out.
"""

import contextlib

import numpy as np

T, D = 2048, 1024
NH, HD = 16, 64
HPC = 4  # heads per core
NCORES = 8
ND = D // 128  # 8 d-tiles
NT = T // 128  # 16 t/k-tiles
NQ = T // 512  # 4 q-chunks

_NC = None


def _build_nc():
    import concourse.mybir as mybir
    import concourse.tile as tile
    from concourse import bacc
    from concourse.masks import make_upper_triangular

    f32 = mybir.dt.float32
    bf16 = mybir.dt.bfloat16
    fp16 = mybir.dt.float16
    Exp = mybir.ActivationFunctionType.Exp

    nc = bacc.Bacc("TRN2", target_bir_lowering=False, debug=False, num_devices=NCORES)

    xT_d = nc.dram_tensor("xT", [D, T], bf16, kind="ExternalInput").ap()
    wq_d = nc.dram_tensor("wq", [D, HPC * HD], bf16, kind="ExternalInput").ap()
    wk_d = nc.dram_tensor("wk", [D, HPC * HD], bf16, kind="ExternalInput").ap()
    wv_d = nc.dram_tensor("wv", [D, HPC * HD], bf16, kind="ExternalInput").ap()
    wo_d = nc.dram_tensor("wo", [HPC * HD, D], bf16, kind="ExternalInput").ap()
    out_d = nc.dram_tensor("out", [T, D], fp16, kind="ExternalOutput").ap()
    rscr = nc.dram_tensor("rscr", [128, 64], f32).ap()
    rscr2 = nc.dram_tensor("rscr2", [128, 64], f32).ap()

    with tile.TileContext(nc) as tc, contextlib.ExitStack() as ctx:
        pool = lambda **kw: ctx.enter_context(tc.tile_pool(**kw))
        constp = pool(name="const", bufs=1)
        qkp = pool(name="qk", bufs=1)
        vp = pool(name="vpool", bufs=1)
        wop = pool(name="wop", bufs=1)
        etp = pool(name="et", bufs=2)
        stgp = pool(name="stg", bufs=1)
        ctgp = pool(name="ctg", bufs=1)
        normp = pool(name="norm", bufs=1)
        rbp = pool(name="rb", bufs=6)
        bctx = contextlib.ExitStack()
        psST = bctx.enter_context(tc.tile_pool(name="psST", bufs=2, space="PSUM"))
        psCT = bctx.enter_context(tc.tile_pool(name="psCT", bufs=2, space="PSUM"))
        actx = contextlib.ExitStack()
        apool = lambda **kw: actx.enter_context(tc.tile_pool(**kw))
        xtp = apool(name="xtr", bufs=1)
        wtp = apool(name="wtiles", bufs=1)
        psProj = apool(name="psProj", bufs=2, space="PSUM")

        mask = constp.tile([128, 128], bf16, name="mask")
        make_upper_triangular(nc, mask[:], val=1.0, diag=True)

        QT = [qkp.tile([128, T], bf16, name=f"QT{g}") for g in range(2)]
        KT = [qkp.tile([128, T], bf16, name=f"KT{g}") for g in range(2)]
        vsb = [vp.tile([128, 66 * HPC], bf16, name=f"v{tt}") for tt in range(NT)]
        wo_sb = [wop.tile([128, D], bf16, name=f"wo{gi}") for gi in range(2)]

        # ---------- loads (bf16 straight from DRAM, few big DMAs) ----------
        wtiles = {}
        wsb = {}

        def load_w(wname, wd):
            wsb[wname] = wtp.tile([128, ND * 256], bf16, name=f"{wname}sb", tag=f"{wname}sb")
            nc.sync.dma_start(wsb[wname][:], wd.rearrange("(a p) c -> p a c", p=128))
            wtiles[wname] = [wsb[wname][:, 256 * dt : 256 * (dt + 1)] for dt in range(ND)]

        load_w("wq", wq_d)
        xtr = [xtp.tile([128, T], bf16, name=f"xtr{dt}", tag=f"xtr{dt}") for dt in range(ND)]
        for dt in range(ND):  # first halves: unblock QT/KT chunks 0-1
            nc.sync.dma_start(xtr[dt][:, 0:1024], xT_d[128 * dt : 128 * (dt + 1), 0:1024])
        load_w("wk", wk_d)
        for dt in range(ND):
            nc.sync.dma_start(xtr[dt][:, 1024:T], xT_d[128 * dt : 128 * (dt + 1), 1024:T])
        load_w("wv", wv_d)
        for gi in range(2):
            nc.sync.dma_start(wo_sb[gi][:], wo_d[128 * gi : 128 * (gi + 1), :])

        # ---------- emission units ----------
        def emit_qkt_unit(wname, outs, g, c):
            ps = psProj.tile([128, 512], f32, name=f"pj_{wname}{g}_{c}", tag="proj")
            for dt in range(ND):
                nc.tensor.matmul(
                    ps[:],
                    wtiles[wname][dt][:, 128 * g : 128 * (g + 1)],
                    xtr[dt][:, 512 * c : 512 * (c + 1)],
                    start=(dt == 0),
                    stop=(dt == ND - 1),
                )
            nc.vector.tensor_copy(outs[g][:, 512 * c : 512 * (c + 1)], ps[:])

        def emit_v(tt):
            ps = psProj.tile([128, 256], f32, name=f"vps{tt}", tag="proj")
            for dt in range(ND):
                nc.tensor.matmul(
                    ps[:],
                    xtr[dt][:, 128 * tt : 128 * (tt + 1)],
                    wtiles["wv"][dt][:],
                    start=(dt == 0),
                    stop=(dt == ND - 1),
                )
            nc.any.memset(vsb[tt][:, 64 : 66 * HPC : 66], 1.0)
            for h in range(HPC):
                nc.vector.tensor_copy(vsb[tt][:, 66 * h : 66 * h + 64], ps[:, 64 * h : 64 * (h + 1)])

        ets = {}  # (h, kt) -> ET tile

        def emit_st(h, kt):
            g = h // 2
            p0 = 64 * (h % 2)
            w = T - 128 * kt
            et = etp.tile([128, w], bf16, name=f"et_h{h}_kt{kt}", tag=f"et{kt}")
            ets[(h, kt)] = et
            for sub in range((w + 1023) // 1024):
                sw = min(1024, w - 1024 * sub)
                q0 = 128 * kt + 1024 * sub
                ps = psST.tile([128, sw], f32, name=f"st_h{h}_k{kt}_s{sub}", tag="st")
                for c in range((sw + 511) // 512):
                    n = min(512, sw - 512 * c)
                    nc.tensor.matmul(
                        ps[:, 512 * c : 512 * c + n],
                        KT[g][p0 : p0 + 64, 128 * kt : 128 * (kt + 1)],
                        QT[g][p0 : p0 + 64, q0 + 512 * c : q0 + 512 * c + n],
                        start=True,
                        stop=True,
                    )
                nc.scalar.activation(
                    et[:, 1024 * sub : 1024 * sub + sw], ps[:, 0:sw], Exp, scale=0.125
                )
            nc.gpsimd.tensor_mul(et[:, 0:128], et[:, 0:128], mask[:])

        stg = {}
        ct_ps = {}

        def emit_ct_mms(h, j, kts, first, last):
            if first:
                ct_ps[(h, j)] = psCT.tile([65, 512], f32, name=f"ct_h{h}_j{j}", tag="ct")
            ct = ct_ps[(h, j)]
            nkt = 4 * j + 4
            for kt in kts:
                etoff = 512 * j - 128 * kt
                if etoff >= 0:
                    n, psoff, ecol = 512, 0, etoff
                else:
                    n, psoff, ecol = 512 + etoff, -etoff, 0
                nc.tensor.matmul(
                    ct[0:65, psoff : psoff + n],
                    vsb[kt][:, 66 * h : 66 * h + 65],
                    ets[(h, kt)][:, ecol : ecol + n],
                    start=(kt == 0),
                    stop=(last and kt == kts[-1]),
                )

        def finish_ct(h, j):
            ct = ct_ps[(h, j)]
            s = stgp.tile([65, 512], f32, name=f"stg_h{h}_j{j}")
            stg[(h, j)] = s
            nc.vector.tensor_copy(s[:], ct[:])
            idx = 4 * h + j
            nc.sync.dma_start(rscr[8 * idx : 8 * idx + 8, :], s[64:65, :])

        # normalize pools created up top (before xtr) for stack order
        CTG = [ctgp.tile([128, T], bf16, name=f"ctg{gi}") for gi in range(2)]
        rscr2v = rscr2.rearrange("(r p) c -> r (p c)", p=8)  # [16, 512] view

        def emit_norm(h, j):
            g, half = h // 2, h % 2
            idx = 4 * h + j
            rs_hj = normp.tile([8, 64], f32, name=f"rs{idx}", tag="rs")
            nc.sync.dma_start(rs_hj[:], rscr[8 * idx : 8 * idx + 8, :])
            rc_hj = normp.tile([8, 64], f32, name=f"rc{idx}", tag="rc")
            nc.vector.reciprocal(rc_hj[:], rs_hj[:])
            nc.sync.dma_start(rscr2[8 * idx : 8 * idx + 8, :], rc_hj[:])
            rb = rbp.tile([64, 512], f32, name=f"rb{idx}", tag="rb")
            nc.sync.dma_start(rb[:], rscr2v[idx : idx + 1, :].partition_broadcast(64))
            eng = nc.vector if j % 2 == 0 else nc.gpsimd
            eng.tensor_mul(
                CTG[g][64 * half : 64 * half + 64, 512 * j : 512 * (j + 1)],
                stg[(h, j)][0:64, :],
                rb[:],
            )

        # ---------- schedule ----------
        emit_qkt_unit("wq", QT, 0, 0)
        emit_qkt_unit("wq", QT, 0, 1)
        emit_qkt_unit("wk", KT, 0, 0)
        emit_qkt_unit("wq", QT, 0, 2)
        emit_qkt_unit("wq", QT, 0, 3)

        # head-serial pipeline. Within each head's 16-slot score window,
        # context-matmul groups are dribbled a few MMs per slot so exp is
        # never starved behind a lumpy PE queue: group j<3 spreads over slots
        # 4j+4..4j+7 (j+1 MMs/slot); group j=3 takes one MM per slot from
        # slot 1 (k-tile kt-1), finishing in-window. Each group's normalize
        # chain (rowsum DMA -> packed reciprocal -> DRAM-bounce broadcast ->
        # multiply into CTG) is emitted at its stop slot.
        dribble = {sw: [] for sw in range(NT)}
        for j in range(3):
            kts = list(range(4 * j + 4))
            for sl in range(4):
                chunk = kts[(j + 1) * sl : (j + 1) * (sl + 1)]
                dribble[4 * j + 4 + sl].append((j, chunk, sl == 0, sl == 3))
        for sl in range(1, 15):
            dribble[sl].append((3, [sl - 1], sl == 1, False))
        dribble[15].append((3, [14, 15], False, True))

        for h in range(HPC):
            for sw in range(NT):
                emit_st(h, sw)
                if h == 0:
                    emit_v(sw)
                    if 1 <= sw <= 3:
                        emit_qkt_unit("wk", KT, 0, sw)
                elif h == 1 and sw % 2 == 0:
                    wn, g1c = ("wq", sw // 4) if sw % 4 == 0 else ("wk", sw // 4)
                    emit_qkt_unit(wn, QT if wn == "wq" else KT, 1, g1c)
                for j, kts_, first, last in dribble[sw]:
                    emit_ct_mms(h, j, kts_, first, last)
                    if last:
                        finish_ct(h, j)
                        emit_norm(h, j)
                if h == 1 and sw == NT - 1:
                    actx.close()
        bctx.close()

        # ---------- output projection ----------
        with (
            tc.tile_pool(name="oh", bufs=4) as ohp,
            tc.tile_pool(name="psO", bufs=2, space="PSUM") as psO,
        ):
            for tt in range(NT):
                ps = psO.tile([128, D], f32, name=f"ops{tt}", tag="ops")
                for gi in range(2):
                    for dc in range(2):
                        nc.tensor.matmul(
                            ps[:, 512 * dc : 512 * (dc + 1)],
                            CTG[gi][:, 128 * tt : 128 * (tt + 1)],
                            wo_sb[gi][:, 512 * dc : 512 * (dc + 1)],
                            start=(gi == 0),
                            stop=(gi == 1),
                        )
                oh = ohp.tile([128, D], fp16, name=f"oh{tt}", tag="oh")
                nc.vector.tensor_copy(oh[:], ps[:])
                nc.sync.dma_start(out_d[128 * tt : 128 * (tt + 1), :], oh[:])

    nc.compile()
    return nc


def _get_nc():
    global _NC
    if _NC is None:
        _NC = _build_nc()
    return _NC


def make_in_maps(x, wq, wk, wv, wo):
    import ml_dtypes

    bf = ml_dtypes.bfloat16
    in_maps = []
    for c in range(NCORES):
        b, g4 = c // 4, c % 4
        cs = slice(256 * g4, 256 * (g4 + 1))
        in_maps.append(
            {
                "xT": np.ascontiguousarray(x[b].T).astype(bf),
                "wq": np.ascontiguousarray(wq[:, cs]).astype(bf),
                "wk": np.ascontiguousarray(wk[:, cs]).astype(bf),
                "wv": np.ascontiguousarray(wv[:, cs]).astype(bf),
                "wo": np.ascontiguousarray(wo[cs, :]).astype(bf),
            }
        )
    return in_maps


def kernel(x, wq, wk, wv, wo, bo):
    from concourse.bass_utils import run_bass_kernel_spmd

    x = np.asarray(x, dtype=np.float32)
    wq = np.asarray(wq, dtype=np.float32)
    wk = np.asarray(wk, dtype=np.float32)
    wv = np.asarray(wv, dtype=np.float32)
    wo = np.asarray(wo, dtype=np.float32)
    bo = np.asarray(bo, dtype=np.float32)

    nc = _get_nc()
    in_maps = make_in_maps(x, wq, wk, wv, wo)
    try:
        res = run_bass_kernel_spmd(nc, in_maps, core_ids=list(range(NCORES))).results
    except Exception:
        # transient NRT device errors have been observed once after a fresh
        # compile; one retry recovers
        res = run_bass_kernel_spmd(nc, in_maps, core_ids=list(range(NCORES))).results
    out = np.zeros((2, T, D), dtype=np.float32)
    for c in range(NCORES):
        out[c // 4] += res[c]["out"].astype(np.float32)
    out += bo[None, None, :]
    return out


# revision 34
# speedup vs baseline: 1.0303x; 1.0069x over previous
"""Multi-head causal attention (B=2, T=2048, D=1024, H=16, HD=64) on 8 TRN2
NeuronCores.

Sharding: batch x head-group. Core c handles batch c//4 and heads
[4*(c%4), 4*(c%4)+4). Wq/Wk/Wv are split column-wise, Wo row-wise; each core
produces a full [T, D] partial output (its 4 heads' contribution, after
per-head softmax normalization and its Wo row-block), which the host sums
across the 4 cores of each batch and adds the bias to.

Per-core kernel (all matmuls contract along SBUF partitions; operands bf16,
accumulation fp32 in PSUM):
  xT [D, T] bf16 (host passes x[b].T pre-cast), weights bf16.
  QT/KT computed transposed [2*64hd, T] per head-pair (lhsT = w, rhs = xT).
  V computed natural [T, 4*64hd] (lhsT = xT, rhs = wv), stored bf16 with a
  ones-column per head (stride 66) so the P@V matmul also produces the
  softmax row-sums (M = 65).
  Scores are computed transposed, ST[k, q] (lhsT = KT, rhs = QT), exact-causal
  (q >= 128*kt per k-tile), exp'd on ACT (scale=1/8 fused) to bf16 ET tiles;
  the strictly-lower triangle of the leading 128x128 diagonal block is zeroed
  with a multiplicative mask (gpsimd). CT' = V'.T @ ET accumulates [65, 512]
  per q-chunk in PSUM; partition 64 is the softmax denominator (exact fp32).
  Reciprocal row-sums are broadcast across partitions via a DRAM bounce and
  multiplied into packed bf16 CT_g [128c, T] tiles (DVE), then
  out[t, do] = CT_g.T @ wo (fp16 partial) is DMA'd out.

Emission order keeps the PE dense (HAM warm): QT/KT(g0) upfront, then the
head-serial pipeline ST(h) || CT(h-1), with QT/KT(g1) and V projection units
spread through head 0's score window; CT(h3) trails, then normalize + out.
"""

import contextlib

import numpy as np

T, D = 2048, 1024
NH, HD = 16, 64
HPC = 4  # heads per core
NCORES = 8
ND = D // 128  # 8 d-tiles
NT = T // 128  # 16 t/k-tiles
NQ = T // 512  # 4 q-chunks

_NC = None


def _build_nc():
    import concourse.mybir as mybir
    import concourse.tile as tile
    from concourse import bacc
    from concourse.masks import make_upper_triangular

    f32 = mybir.dt.float32
    bf16 = mybir.dt.bfloat16
    fp16 = mybir.dt.float16
    Exp = mybir.ActivationFunctionType.Exp

    nc = bacc.Bacc("TRN2", target_bir_lowering=False, debug=False, num_devices=NCORES)

    xT_d = nc.dram_tensor("xT", [D, T], bf16, kind="ExternalInput").ap()
    wq_d = nc.dram_tensor("wq", [D, HPC * HD], bf16, kind="ExternalInput").ap()
    wk_d = nc.dram_tensor("wk", [D, HPC * HD], bf16, kind="ExternalInput").ap()
    wv_d = nc.dram_tensor("wv", [D, HPC * HD], bf16, kind="ExternalInput").ap()
    wo_d = nc.dram_tensor("wo", [HPC * HD, D], bf16, kind="ExternalInput").ap()
    out_d = nc.dram_tensor("out", [T, D], fp16, kind="ExternalOutput").ap()
    rscr = nc.dram_tensor("rscr", [128, 64], f32).ap()
    rscr2 = nc.dram_tensor("rscr2", [128, 64], f32).ap()

    with tile.TileContext(nc) as tc, contextlib.ExitStack() as ctx:
        pool = lambda **kw: ctx.enter_context(tc.tile_pool(**kw))
        constp = pool(name="const", bufs=1)
        qkp = pool(name="qk", bufs=1)
        vp = pool(name="vpool", bufs=1)
        wop = pool(name="wop", bufs=1)
        etp = pool(name="et", bufs=2)
        stgp = pool(name="stg", bufs=1)
        ctgp = pool(name="ctg", bufs=1)
        normp = pool(name="norm", bufs=4)
        rbp = pool(name="rb", bufs=8)
        bctx = contextlib.ExitStack()
        psST = bctx.enter_context(tc.tile_pool(name="psST", bufs=2, space="PSUM"))
        psCT = bctx.enter_context(tc.tile_pool(name="psCT", bufs=2, space="PSUM"))
        actx = contextlib.ExitStack()
        apool = lambda **kw: actx.enter_context(tc.tile_pool(**kw))
        xtp = apool(name="xtr", bufs=1)
        wtp = apool(name="wtiles", bufs=1)
        psProj = apool(name="psProj", bufs=2, space="PSUM")

        mask = constp.tile([128, 128], bf16, name="mask")
        make_upper_triangular(nc, mask[:], val=1.0, diag=True)

        QT = [qkp.tile([128, T], bf16, name=f"QT{g}") for g in range(2)]
        KT = [qkp.tile([128, T], bf16, name=f"KT{g}") for g in range(2)]
        vsb = [vp.tile([128, 66 * HPC], bf16, name=f"v{tt}") for tt in range(NT)]
        wo_sb = [wop.tile([128, D], bf16, name=f"wo{gi}") for gi in range(2)]

        # ---------- loads (bf16 straight from DRAM, few big DMAs) ----------
        wtiles = {}
        wsb = {}

        def load_w(wname, wd):
            wsb[wname] = wtp.tile([128, ND * 256], bf16, name=f"{wname}sb", tag=f"{wname}sb")
            nc.sync.dma_start(wsb[wname][:], wd.rearrange("(a p) c -> p a c", p=128))
            wtiles[wname] = [wsb[wname][:, 256 * dt : 256 * (dt + 1)] for dt in range(ND)]

        load_w("wq", wq_d)
        xtr = [xtp.tile([128, T], bf16, name=f"xtr{dt}", tag=f"xtr{dt}") for dt in range(ND)]
        for dt in range(ND):  # first halves: unblock QT/KT chunks 0-1
            nc.sync.dma_start(xtr[dt][:, 0:1024], xT_d[128 * dt : 128 * (dt + 1), 0:1024])
        load_w("wk", wk_d)
        for dt in range(ND):
            nc.sync.dma_start(xtr[dt][:, 1024:T], xT_d[128 * dt : 128 * (dt + 1), 1024:T])
        load_w("wv", wv_d)
        for gi in range(2):
            nc.sync.dma_start(wo_sb[gi][:], wo_d[128 * gi : 128 * (gi + 1), :])

        # ---------- emission units ----------
        def emit_qkt_unit(wname, outs, g, c):
            ps = psProj.tile([128, 512], f32, name=f"pj_{wname}{g}_{c}", tag="proj")
            for dt in range(ND):
                nc.tensor.matmul(
                    ps[:],
                    wtiles[wname][dt][:, 128 * g : 128 * (g + 1)],
                    xtr[dt][:, 512 * c : 512 * (c + 1)],
                    start=(dt == 0),
                    stop=(dt == ND - 1),
                )
            nc.vector.tensor_copy(outs[g][:, 512 * c : 512 * (c + 1)], ps[:])

        def emit_v(tt):
            ps = psProj.tile([128, 256], f32, name=f"vps{tt}", tag="proj")
            for dt in range(ND):
                nc.tensor.matmul(
                    ps[:],
                    xtr[dt][:, 128 * tt : 128 * (tt + 1)],
                    wtiles["wv"][dt][:],
                    start=(dt == 0),
                    stop=(dt == ND - 1),
                )
            nc.any.memset(vsb[tt][:, 64 : 66 * HPC : 66], 1.0)
            for h in range(HPC):
                nc.vector.tensor_copy(vsb[tt][:, 66 * h : 66 * h + 64], ps[:, 64 * h : 64 * (h + 1)])

        ets = {}  # (h, kt) -> ET tile

        def emit_st(h, kt):
            g = h // 2
            p0 = 64 * (h % 2)
            w = T - 128 * kt
            et = etp.tile([128, w], bf16, name=f"et_h{h}_kt{kt}", tag=f"et{kt}")
            ets[(h, kt)] = et
            for sub in range((w + 1023) // 1024):
                sw = min(1024, w - 1024 * sub)
                q0 = 128 * kt + 1024 * sub
                ps = psST.tile([128, sw], f32, name=f"st_h{h}_k{kt}_s{sub}", tag="st")
                for c in range((sw + 511) // 512):
                    n = min(512, sw - 512 * c)
                    nc.tensor.matmul(
                        ps[:, 512 * c : 512 * c + n],
                        KT[g][p0 : p0 + 64, 128 * kt : 128 * (kt + 1)],
                        QT[g][p0 : p0 + 64, q0 + 512 * c : q0 + 512 * c + n],
                        start=True,
                        stop=True,
                    )
                nc.scalar.activation(
                    et[:, 1024 * sub : 1024 * sub + sw], ps[:, 0:sw], Exp, scale=0.125
                )
            nc.gpsimd.tensor_mul(et[:, 0:128], et[:, 0:128], mask[:])

        stg = {}
        ct_ps = {}

        def emit_ct_mms(h, j, kts, first, last):
            if first:
                ct_ps[(h, j)] = psCT.tile([65, 512], f32, name=f"ct_h{h}_j{j}", tag="ct")
            ct = ct_ps[(h, j)]
            nkt = 4 * j + 4
            for kt in kts:
                etoff = 512 * j - 128 * kt
                if etoff >= 0:
                    n, psoff, ecol = 512, 0, etoff
                else:
                    n, psoff, ecol = 512 + etoff, -etoff, 0
                nc.tensor.matmul(
                    ct[0:65, psoff : psoff + n],
                    vsb[kt][:, 66 * h : 66 * h + 65],
                    ets[(h, kt)][:, ecol : ecol + n],
                    start=(kt == 0),
                    stop=(last and kt == kts[-1]),
                )

        def finish_ct(h, j):
            ct = ct_ps[(h, j)]
            s = stgp.tile([65, 512], f32, name=f"stg_h{h}_j{j}")
            stg[(h, j)] = s
            nc.vector.tensor_copy(s[:], ct[:])
            idx = 4 * h + j
            nc.sync.dma_start(rscr[8 * idx : 8 * idx + 8, :], s[64:65, :])

        # normalize pools created up top (before xtr) for stack order
        CTG = [ctgp.tile([128, T], bf16, name=f"ctg{gi}") for gi in range(2)]
        rscr2v = rscr2.rearrange("(r p) c -> r (p c)", p=8)  # [16, 512] view

        def emit_norm(h, j):
            g, half = h // 2, h % 2
            idx = 4 * h + j
            rs_hj = normp.tile([8, 64], f32, name=f"rs{idx}", tag="rs")
            nc.sync.dma_start(rs_hj[:], rscr[8 * idx : 8 * idx + 8, :])
            rc_hj = normp.tile([8, 64], f32, name=f"rc{idx}", tag="rc")
            nc.vector.reciprocal(rc_hj[:], rs_hj[:])
            nc.sync.dma_start(rscr2[8 * idx : 8 * idx + 8, :], rc_hj[:])
            rb = rbp.tile([64, 512], f32, name=f"rb{idx}", tag="rb")
            nc.sync.dma_start(rb[:], rscr2v[idx : idx + 1, :].partition_broadcast(64))
            eng = nc.vector if j % 2 == 0 else nc.gpsimd
            eng.tensor_mul(
                CTG[g][64 * half : 64 * half + 64, 512 * j : 512 * (j + 1)],
                stg[(h, j)][0:64, :],
                rb[:],
            )

        # ---------- schedule ----------
        emit_qkt_unit("wq", QT, 0, 0)
        emit_qkt_unit("wq", QT, 0, 1)
        emit_qkt_unit("wk", KT, 0, 0)
        emit_qkt_unit("wq", QT, 0, 2)
        emit_qkt_unit("wq", QT, 0, 3)

        # head-serial pipeline. Within each head's 16-slot score window,
        # context-matmul groups are dribbled a few MMs per slot so exp is
        # never starved behind a lumpy PE queue: group j<3 spreads over slots
        # 4j+4..4j+7 (j+1 MMs/slot); group j=3 takes one MM per slot from
        # slot 1 (k-tile kt-1), finishing in-window. Each group's normalize
        # chain (rowsum DMA -> packed reciprocal -> DRAM-bounce broadcast ->
        # multiply into CTG) is emitted at its stop slot.
        dribble = {sw: [] for sw in range(NT)}
        for j in range(3):
            kts = list(range(4 * j + 4))
            for sl in range(4):
                chunk = kts[(j + 1) * sl : (j + 1) * (sl + 1)]
                dribble[4 * j + 4 + sl].append((j, chunk, sl == 0, sl == 3))
        for sl in range(1, 15):
            dribble[sl].append((3, [sl - 1], sl == 1, False))
        dribble[15].append((3, [14, 15], False, True))

        for h in range(HPC):
            for sw in range(NT):
                emit_st(h, sw)
                if h == 0:
                    emit_v(sw)
                    if 1 <= sw <= 3:
                        emit_qkt_unit("wk", KT, 0, sw)
                elif h == 1 and sw % 2 == 0:
                    wn, g1c = ("wq", sw // 4) if sw % 4 == 0 else ("wk", sw // 4)
                    emit_qkt_unit(wn, QT if wn == "wq" else KT, 1, g1c)
                for j, kts_, first, last in dribble[sw]:
                    emit_ct_mms(h, j, kts_, first, last)
                    if last:
                        finish_ct(h, j)
                        emit_norm(h, j)
                if h == 1 and sw == NT - 1:
                    actx.close()
        bctx.close()

        # ---------- output projection ----------
        with (
            tc.tile_pool(name="oh", bufs=4) as ohp,
            tc.tile_pool(name="psO", bufs=2, space="PSUM") as psO,
        ):
            for tt in range(NT):
                ps = psO.tile([128, D], f32, name=f"ops{tt}", tag="ops")
                for gi in range(2):
                    for dc in range(2):
                        nc.tensor.matmul(
                            ps[:, 512 * dc : 512 * (dc + 1)],
                            CTG[gi][:, 128 * tt : 128 * (tt + 1)],
                            wo_sb[gi][:, 512 * dc : 512 * (dc + 1)],
                            start=(gi == 0),
                            stop=(gi == 1),
                        )
                oh = ohp.tile([128, D], fp16, name=f"oh{tt}", tag="oh")
                nc.vector.tensor_copy(oh[:], ps[:])
                nc.sync.dma_start(out_d[128 * tt : 128 * (tt + 1), :], oh[:])

    nc.compile()
    return nc


def _get_nc():
    global _NC
    if _NC is None:
        _NC = _build_nc()
    return _NC


def make_in_maps(x, wq, wk, wv, wo):
    import ml_dtypes

    bf = ml_dtypes.bfloat16
    in_maps = []
    for c in range(NCORES):
        b, g4 = c // 4, c % 4
        cs = slice(256 * g4, 256 * (g4 + 1))
        in_maps.append(
            {
                "xT": np.ascontiguousarray(x[b].T).astype(bf),
                "wq": np.ascontiguousarray(wq[:, cs]).astype(bf),
                "wk": np.ascontiguousarray(wk[:, cs]).astype(bf),
                "wv": np.ascontiguousarray(wv[:, cs]).astype(bf),
                "wo": np.ascontiguousarray(wo[cs, :]).astype(bf),
            }
        )
    return in_maps


def kernel(x, wq, wk, wv, wo, bo):
    from concourse.bass_utils import run_bass_kernel_spmd

    x = np.asarray(x, dtype=np.float32)
    wq = np.asarray(wq, dtype=np.float32)
    wk = np.asarray(wk, dtype=np.float32)
    wv = np.asarray(wv, dtype=np.float32)
    wo = np.asarray(wo, dtype=np.float32)
    bo = np.asarray(bo, dtype=np.float32)

    nc = _get_nc()
    in_maps = make_in_maps(x, wq, wk, wv, wo)
    try:
        res = run_bass_kernel_spmd(nc, in_maps, core_ids=list(range(NCORES))).results
    except Exception:
        # transient NRT device errors have been observed once after a fresh
        # compile; one retry recovers
        res = run_bass_kernel_spmd(nc, in_maps, core_ids=list(range(NCORES))).results
    out = np.zeros((2, T, D), dtype=np.float32)
    for c in range(NCORES):
        out[c // 4] += res[c]["out"].astype(np.float32)
    out += bo[None, None, :]
    return out
